# revision 1
# baseline (speedup 1.0000x reference)
"""CML2DWithStats Trainium2 kernel.

15-step coupled-map-lattice: g' = 0.595*m + 0.255*conv3x3(m) + 0.15*drive,
m = R*g*(1-g), clamp (never binds, verified margin >0.05), over
[16,8,256,256] f32, returning (last, mean, var, delta, delta).

Reformulation: with s = (g-1/2)^2 and a = R/4:
    m = a - R*s
    g' = D - sum_{dy,dx} W[dy,dx] * shift_{dy,dx}(s)        (s zero-padded)
    D  = 0.595*a + 0.255*a*C0 + 0.15*drive   (C0 = in-bounds kernel sum)
    W  = R*(0.255*k + 0.595*center)
    var = mean_t[(g_t-1/2)^2] - (mean_g - 1/2)^2  (translation invariance)

Data parallel across 8 NeuronCores (2 batch samples each). Per-core layout:
128 partitions = 8 row-chunks x 16 images (2 samples x 8 channels); each
partition holds a [34 rows x 258 cols] zero-padded slab of one image chunk
(32 owned rows + 2 halo rows, 256 cols + 2 pad cols) so the whole 3x3
stencil is 9 free-dim-offset FMAs (scalar_tensor_tensor with per-partition
weight APs). Halo rows are refreshed by 2 partition-shifted SBUF-SBUF DMAs
per step. ACT computes s' = (g'-1/2)^2 (chunk-edge rows first so next
step's halo DMAs launch early); running sums gsum and ssum accumulate on
GPSIMD in parallel with the DVE tap chain (g is double-buffered across
steps so GPSIMD's read never stalls the next step's taps). mean/var/delta
finalization is trivial elementwise postprocessing done on the host.

Raw Bass (no TileContext): this toolchain's walrus rejects instructions
carrying more than one inline sync-wait, so all cross-engine ordering uses
standalone wait_ge instructions with hand-counted semaphore targets.
"""

import sys

sys.path.insert(0, "/opt/trn_rl_repo")

import numpy as np

R_PARAM = np.float32(3.9)
EPS = np.float32(0.3)
BETA = np.float32(0.15)
STEPS = 15
A = np.float32(R_PARAM / 4.0)  # 0.975

B, C, H, W = 16, 8, 256, 256
N_CORES = 8
BL = B // N_CORES  # 2 samples per core

P = 128
NJ, NI, CH = 8, 16, 32  # chunks, images/core, rows/chunk
ROWS, COLS = CH + 2, W + 2  # 34, 258
S_FREE = ROWS * COLS + 2  # 8774 (1 lead + 1 tail pad elem)
G_FREE = CH * COLS  # 8256
PK_FREE = CH * W  # 8192

# dy=0 taps first (only owned rows -> no halo dependency), then dy=+-1
TAPS = [(0, -1), (0, 0), (0, 1),
        (-1, -1), (-1, 0), (-1, 1), (1, -1), (1, 0), (1, 1)]

_CACHE = {}


def _build_program():
    import concourse.bass as bass
    import concourse.mybir as mybir

    dt = mybir.dt
    f32 = dt.float32
    Alu = mybir.AluOpType

    nc = bass.Bass()

    inp_d = nc.dram_tensor("inp", [P, S_FREE + G_FREE + 16], f32, kind="ExternalInput")
    last_d = nc.dram_tensor("last", [P, G_FREE], f32, kind="ExternalOutput")
    gsum_d = nc.dram_tensor("gsum", [P, G_FREE], f32, kind="ExternalOutput")
    ssum_d = nc.dram_tensor("ssum", [P, PK_FREE], f32, kind="ExternalOutput")

    taps = TAPS

    base_t = nc.alloc_sbuf_tensor("base", [P, S_FREE + G_FREE + 16], f32)
    g_t = nc.alloc_sbuf_tensor("g", [P, G_FREE], f32)
    gB_t = nc.alloc_sbuf_tensor("gB", [P, G_FREE], f32)
    gsum_t = nc.alloc_sbuf_tensor("gsumb", [P, G_FREE], f32)
    ssum_t = nc.alloc_sbuf_tensor("ssumb", [P, PK_FREE], f32)

    base = base_t.ap()
    s_all = base[:, 0:S_FREE]
    D = base[:, S_FREE : S_FREE + G_FREE]
    wv = base[:, S_FREE + G_FREE : S_FREE + G_FREE + 16]
    g = g_t.ap()
    gB = gB_t.ap()
    gbuf = [g, gB]
    gsum = gsum_t.ap()
    ssum = ssum_t.ap()

    def s_row_core(rr):
        o = 1 + rr * COLS + 1
        return base[:, o : o + W]

    s_own = (
        base[:, 1 + COLS : 1 + COLS + CH * COLS]
        .rearrange("p (r x) -> p r x", x=COLS)[:, :, 1 : 1 + W]
    )
    ssum_v = ssum.rearrange("p (r x) -> p r x", x=W)

    T = STEPS  # 15

    # edge rows of the owned region (image rows at chunk borders)
    g_row0 = [gg[:, 1 : 1 + W] for gg in gbuf]
    g_row31 = [gg[:, 31 * COLS + 1 : 31 * COLS + 1 + W] for gg in gbuf]
    s_int = (
        base[:, 1 + 2 * COLS : 1 + 2 * COLS + 30 * COLS]
        .rearrange("p (r x) -> p r x", x=COLS)[:, :, 1 : 1 + W]
    )
    g_int = [
        gg[:, COLS : COLS + 30 * COLS]
        .rearrange("p (r x) -> p r x", x=COLS)[:, :, 1 : 1 + W]
        for gg in gbuf
    ]

    with (
        nc.semaphore() as inp_sem,
        nc.semaphore() as dma_sem,
        nc.semaphore() as dve_sem,
        nc.semaphore() as act_sem,
        nc.semaphore() as acte_sem,
        nc.semaphore() as pool_sem,
        nc.semaphore() as gpg_sem,
        nc.Block() as block,
    ):

        @block.sync
        def _(sync):
            nc.sync.dma_start(base, inp_d[:]).then_inc(inp_sem, 16)
            for t in range(1, T):
                # halos for step t need: s'(t-1) edge rows written, and all
                # step-(t-1) tap reads of the old halo rows retired.
                nc.sync.wait_ge(acte_sem, 2 * t)
                nc.sync.wait_ge(dve_sem, 2 * t)
                nc.sync.dma_start(
                    s_row_core(0)[16:128, :], s_row_core(CH)[0:112, :]
                ).then_inc(dma_sem, 16)
                nc.sync.dma_start(
                    s_row_core(ROWS - 1)[0:112, :], s_row_core(1)[16:128, :]
                ).then_inc(dma_sem, 16)
            nc.sync.wait_ge(dve_sem, 2 * (T - 1) + 2)
            nc.sync.dma_start(last_d[:], g).then_inc(dma_sem, 16)
            nc.sync.wait_ge(gpg_sem, T)
            nc.sync.dma_start(gsum_d[:], gsum).then_inc(dma_sem, 16)
            nc.sync.wait_ge(pool_sem, T)
            nc.sync.dma_start(ssum_d[:], ssum).then_inc(dma_sem, 16)
            nc.sync.wait_ge(dma_sem, 32 * (T - 1) + 48)

        @block.vector
        def _(vector):
            nc.vector.wait_ge(inp_sem, 16)
            for t in range(T):
                gc = gbuf[t % 2]
                if t > 0:
                    nc.vector.wait_ge(act_sem, t)
                    nc.vector.wait_ge(acte_sem, 2 * t)
                if t > 1:
                    nc.vector.wait_ge(gpg_sem, t - 1)
                for k, (dy, dx) in enumerate(taps):
                    if t > 0 and k == 3:
                        nc.vector.wait_ge(dma_sem, 32 * t)
                    off = 1 + (1 + dy) * COLS + dx
                    ins = nc.vector.scalar_tensor_tensor(
                        out=gc,
                        in0=base[:, off : off + G_FREE],
                        scalar=wv[:, k : k + 1],
                        in1=(D if k == 0 else gc),
                        op0=Alu.mult,
                        op1=Alu.add,
                    )
                    if k == 8:
                        ins.then_inc(dve_sem, 2)  # -> 2t+2: g final

        @block.scalar
        def _(scalar):
            Sq = mybir.ActivationFunctionType.Square
            for t in range(T):
                gc = t % 2
                nc.scalar.wait_ge(dve_sem, 2 * t + 2)
                if t > 0:
                    nc.scalar.wait_ge(pool_sem, t)
                    # edge squares overwrite rows the step-t halo DMAs read
                    nc.scalar.wait_ge(dma_sem, 32 * t)
                # edge rows first so next step's halo DMAs can start early
                nc.scalar.activation(
                    s_row_core(1), g_row0[gc], Sq, bias=wv[:, 9:10], scale=1.0
                ).then_inc(acte_sem, 1)
                nc.scalar.activation(
                    s_row_core(CH), g_row31[gc], Sq, bias=wv[:, 9:10], scale=1.0
                ).then_inc(acte_sem, 1)  # -> 2t+2
                nc.scalar.activation(
                    s_int, g_int[gc], Sq, bias=wv[:, 9:10], scale=1.0
                ).then_inc(act_sem, 1)  # -> t+1

        @block.gpsimd
        def _(gpsimd):
            nc.gpsimd.memset(ssum, 0.0)
            nc.gpsimd.memset(gsum, 0.0)
            for t in range(T):
                nc.gpsimd.wait_ge(dve_sem, 2 * t + 2)
                nc.gpsimd.tensor_tensor(
                    out=gsum, in0=gsum, in1=gbuf[t % 2], op=Alu.add
                ).then_inc(gpg_sem, 1)
                nc.gpsimd.wait_ge(act_sem, t + 1)
                nc.gpsimd.wait_ge(acte_sem, 2 * t + 2)
                nc.gpsimd.tensor_tensor(
                    out=ssum_v, in0=ssum_v, in1=s_own, op=Alu.add
                ).then_inc(pool_sem, 1)

    return nc


def _get_nc():
    if "nc" not in _CACHE:
        _CACHE["nc"] = _build_program()
    return _CACHE["nc"]


def _conv_inbounds_sum(k):
    """C0[y,x] = sum of kernel taps that land in-bounds (per channel)."""
    c0 = np.zeros((H, W), dtype=np.float64)
    ones = np.ones((H, W), dtype=np.float64)
    pad = np.pad(ones, 1)
    for dy in range(3):
        for dx in range(3):
            c0 += k[dy, dx] * pad[dy : dy + H, dx : dx + W]
    return c0.astype(np.float32)


def _pack_g(x):
    """[BL,C,H,W] -> [P, G_FREE] (owned rows, padded cols, pads zero)."""
    out = np.zeros((NJ, NI, CH, COLS), dtype=np.float32)
    xr = x.reshape(NI, NJ, CH, W)  # i=(s,c) major, then chunk j, row r, col
    # partition p = j*16 + i -> order (j, i)
    out[:, :, :, 1 : 1 + W] = np.transpose(xr, (1, 0, 2, 3))
    return out.reshape(P, G_FREE)


def _unpack_g(y):
    """[P, G_FREE] -> [BL,C,H,W]."""
    yr = y.reshape(NJ, NI, CH, COLS)[:, :, :, 1 : 1 + W]
    return np.transpose(yr, (1, 0, 2, 3)).reshape(BL, C, H, W).copy()


def _unpack_pk(y):
    yr = y.reshape(NJ, NI, CH, W)
    return np.transpose(yr, (1, 0, 2, 3)).reshape(BL, C, H, W).copy()


def _pack_s0(s0_img):
    """[BL,C,H,W] -> [P, S_FREE] with halo rows and zero pads."""
    out = np.zeros((NJ, NI, ROWS, COLS), dtype=np.float32)
    padded = np.zeros((NI, H + 2, W), dtype=np.float32)
    padded[:, 1 : 1 + H, :] = s0_img.reshape(NI, H, W)
    for j in range(NJ):
        # slab rows rr=0..33 <-> image rows 32j-1 .. 32j+32 <-> padded rows 32j..32j+33
        out[j, :, :, 1 : 1 + W] = padded[:, 32 * j : 32 * j + ROWS, :]
    flat = np.zeros((P, S_FREE), dtype=np.float32)
    flat[:, 1 : 1 + ROWS * COLS] = out.reshape(P, ROWS * COLS)
    return flat


def kernel(drive, K_local, trace=False):
    from concourse.bass_utils import run_bass_kernel_spmd

    drive = np.asarray(drive, dtype=np.float32)
    K_local = np.asarray(K_local, dtype=np.float32)
    k = K_local[:, 0]  # [C,3,3]

    nc = _get_nc()

    # per-channel folded stencil weights (negated for the STT accumulate)
    w_full = (np.float32(0.255) * R_PARAM) * k  # [C,3,3]
    w_full[:, 1, 1] += np.float32(0.595) * R_PARAM
    # weight vector per partition: channel of partition p = (p % 16) % 8
    ch_of_p = (np.arange(P) % NI) % C
    w_taps = np.stack(
        [w_full[:, dy + 1, dx + 1] for (dy, dx) in TAPS], axis=1
    )  # [C, 9] in TAPS order
    wv = np.concatenate(
        [-w_taps[ch_of_p], np.full((P, 1), -0.5)], axis=1
    ).astype(np.float32)

    # D field, per channel C0
    c0 = np.stack([_conv_inbounds_sum(k[c].astype(np.float64)) for c in range(C)])
    d_const = (np.float32(0.595) * A) + (np.float32(0.255) * A) * c0[None]  # [1,C,H,W]
    in_maps = []
    for cid in range(N_CORES):
        dcore = drive[BL * cid : BL * (cid + 1)]  # [BL,C,H,W]
        Df = (d_const + BETA * dcore).astype(np.float32)
        s0 = np.square(dcore - np.float32(0.5), dtype=np.float32)
        inp = np.zeros((P, S_FREE + G_FREE + 16), dtype=np.float32)
        inp[:, 0:S_FREE] = _pack_s0(s0)
        inp[:, S_FREE : S_FREE + G_FREE] = _pack_g(Df)
        inp[:, S_FREE + G_FREE : S_FREE + G_FREE + 10] = wv
        in_maps.append({"inp": inp})

    r = run_bass_kernel_spmd(nc, in_maps, list(range(N_CORES)), trace=trace)
    if trace and r.exec_time_ns is not None:
        print(f"HW exec time: {r.exec_time_ns} ns")
        _CACHE["exec_time_ns"] = r.exec_time_ns
        _CACHE["profile"] = r
    res = r.results

    last = np.empty((B, C, H, W), dtype=np.float32)
    mean = np.empty((B, C, H, W), dtype=np.float32)
    var = np.empty((B, C, H, W), dtype=np.float32)
    inv_steps = np.float32(1.0 / STEPS)
    for cid in range(N_CORES):
        sl = slice(BL * cid, BL * (cid + 1))
        last[sl] = _unpack_g(res[cid]["last"])
        gsum = _unpack_g(res[cid]["gsum"])
        ssum = _unpack_pk(res[cid]["ssum"])
        m = gsum * inv_steps
        mean[sl] = m
        var[sl] = ssum * inv_steps - np.square(m - np.float32(0.5), dtype=np.float32)

    delta = last - drive
    return (last, mean, var, delta, delta.copy())



# revision 19
# speedup vs baseline: 5.3763x; 5.3763x over previous
"""CML2DWithStats Trainium2 kernel — PE-array (matmul) stencil, f16 operands.

15-step coupled map lattice on [16,8,256,256] f32:
    m = R*g*(1-g);  g' = clamp(0.595*m + 0.255*conv3x3(m) + 0.15*drive)
returning (last, mean, var, delta, delta).  Clamp never binds.

Reformulation (s := (g-1/2)^2, a := R/4):
    g' - 1/2 = Dt - L(s),  Dt = 0.595*a + 0.255*a*C0 + 0.15*drive - 1/2
    L = 3x3 stencil, weights W = R*(0.255*K + 0.595*center);  s' = (g'-1/2)^2
Stats are affine in accumulated s:  sum_t g_t = 15*Dt - L(AS) + 15/2 with
AS = sum_{t=0..14} s_t;  ssum = AS - s_0 + s_15.  Host finalizes mean/var.

Device mapping (per core; pure data parallel over 8 cores, 2 samples each):
the 16 images' rows form one zero-separated stream of 4113 rows packed into
NB=33 overlapping blocks of 128 rows (126 owned + 1 halo row each side).
Rows live on PARTITIONS, columns on the free axis ([128, 258] f16 regions,
zero pad cols).  Per block per step the stencil is 3 PE matmuls (one per
column shift dx) with banded 128x128 f16 lhsT (row taps, negated,
per-channel), accumulating into a PSUM slot; f16 keeps matmuls at 1 PE
cycle/row and its 10-bit mantissa leaves ~1e-3 final error (gate is 2e-2).
Dt enters via an identity matmul on PE for blocks 0..3 and 20..32, and via
DVE scalar_tensor_tensor psum-adds for blocks 4..19 (load balance: PE would
otherwise bottleneck).  ACT squares PSUM -> s (f16), DVE accumulates AS
(f16, 2x mode), 6 tiny SBUF-SBUF DMAs per step refresh halo partition rows
at three granularities so the next step's PE never stalls.  Step 15: ACT
evacuates psum = last-1/2 to an f32 stage; step 16 computes 15*Dt - L(AS)
on PE (id15 / scalar=15) and ACT evacuates psum/15 into the dead s region.
Host gathers last/mean/AS and finishes mean/var/delta.

Raw Bass; hand-counted semaphore targets; DMA ordering uses one inline
wait per DMA instruction (async transfers ignore seq-level waits) and
role-separated semaphores so thresholds are unambiguous under out-of-order
DMA completion.
"""

import sys

sys.path.insert(0, "/opt/trn_rl_repo")

import numpy as np

R_PARAM = 3.9
STEPS = 15

B, C, H, W = 16, 8, 256, 256
N_CORES = 8
BL = B // N_CORES  # samples per core
NIMG = BL * C  # 16 images per core

NB = 33  # row blocks
OWN = 126  # owned rows per block
RPI = H + 1  # stream rows per image (1 leading Z + 256)
NS = NIMG * RPI + 1  # 4113 stream rows (incl trailing Z)
RW = 258  # region width (256 cols + 2 zero pad)

W_OFF = 0
W_LEN = NB * 3 * 128  # 12672
ID_OFF = W_OFF + W_LEN
ID15_OFF = ID_OFF + 128
DT_OFF = ID15_OFF + 128  # 12928
S_OFF = DT_OFF + NB * RW  # 21442
IN_LEN = S_OFF + NB * RW  # 29956 f16 elements DMAed in
AS_OFF = IN_LEN
BASE_F = AS_OFF + NB * RW  # 38470 f16 per partition
STAGE_F = NB * 256  # 8448 f32 per partition (last)

# identity-matmul blocks on PE; Dt for the rest is added by DVE
ID_BATCH = {0, 5, 6, 7, 8}  # blocks 0..3 and 20..32
# DVE Dt batches: (block range, psum slot base)
D_BATCH = [(4, 8, 4), (8, 12, 8), (12, 16, 12), (16, 20, 0)]

_CACHE = {}


def _build_program():
    import concourse.bass as bass
    import concourse.mybir as mybir

    f32 = mybir.dt.float32
    f16 = mybir.dt.float16
    Alu = mybir.AluOpType
    Act = mybir.ActivationFunctionType

    nc = bass.Bass()

    inp_d = nc.dram_tensor("inp", [128, IN_LEN], f16, kind="ExternalInput")
    last_d = nc.dram_tensor("last_ps", [128, NB * 256], f32, kind="ExternalOutput")
    mean_d = nc.dram_tensor("mean_ps", [128, NB * 256], f16, kind="ExternalOutput")
    asum_d = nc.dram_tensor("asum", [128, NB * RW], f16, kind="ExternalOutput")

    base_t = nc.alloc_sbuf_tensor("base", [128, BASE_F], f16)
    stage_t = nc.alloc_sbuf_tensor("stage", [128, STAGE_F], f32)
    psum_t = nc.alloc_psum_tensor("ps", [128, 4096], f32)

    base = base_t.ap()
    ps = psum_t.ap()

    wv = base[:, W_OFF : W_OFF + W_LEN]
    ident = base[:, ID_OFF : ID_OFF + 128]
    id15 = base[:, ID15_OFF : ID15_OFF + 128]
    dt_r = base[:, DT_OFF : DT_OFF + NB * RW]
    s_r = base[:, S_OFF : S_OFF + NB * RW]
    as_r = base[:, AS_OFF : AS_OFF + NB * RW]
    stage = stage_t.ap()
    mst = s_r[:, 0 : NB * 256]

    dt_3d = dt_r.rearrange("p (b x) -> p b x", x=RW)
    s_3d = s_r.rearrange("p (b x) -> p b x", x=RW)
    as_3d = as_r.rearrange("p (b x) -> p b x", x=RW)
    st_3d = stage.rearrange("p (b x) -> p b x", x=256)
    mst_3d = mst.rearrange("p (b x) -> p b x", x=256)
    ps_3d = ps.rearrange("p (s x) -> p s x", x=256)

    def brange(j):
        return (4 * j, min(4 * j + 4, NB))

    T = STEPS

    with (
        nc.semaphore() as in_sem,
        nc.semaphore() as h0_sem,
        nc.semaphore() as h1_sem,
        nc.semaphore() as h2_sem,
        nc.semaphore() as ash_sem,
        nc.semaphore() as out_sem,
        nc.semaphore() as pe_sem,
        nc.semaphore() as act_sem,
        nc.semaphore() as as_sem,
        nc.semaphore() as d_sem,
        nc.Block() as block,
    ):

        @block.sync
        def _(sync):
            nc.sync.dma_start(base[:, 0:IN_LEN], inp_d[:]).then_inc(in_sem, 16)
            for t in range(1, T):  # halos completing s_t, t = 1..14
                # pair 0: dst regions 0..4 (after ACT(t) blocks 0..7)
                nc.sync.dma_start(
                    s_r[0:1, RW : 5 * RW], s_r[126:127, 0 : 4 * RW]
                )._wait_ge(act_sem, 33 * (t - 1) + 8).then_inc(h0_sem, 16)
                nc.sync.dma_start(
                    s_r[127:128, 0 : 4 * RW], s_r[1:2, RW : 5 * RW]
                )._wait_ge(act_sem, 33 * (t - 1) + 8).then_inc(h0_sem, 16)
                # pair 1: dst regions 4..16 (after ACT(t) blocks 0..19)
                nc.sync.dma_start(
                    s_r[0:1, 5 * RW : 17 * RW], s_r[126:127, 4 * RW : 16 * RW]
                )._wait_ge(act_sem, 33 * (t - 1) + 20).then_inc(h1_sem, 16)
                nc.sync.dma_start(
                    s_r[127:128, 4 * RW : 16 * RW], s_r[1:2, 5 * RW : 17 * RW]
                )._wait_ge(act_sem, 33 * (t - 1) + 20).then_inc(h1_sem, 16)
                # pair 2: dst regions 16..32 (after all ACT(t))
                nc.sync.dma_start(
                    s_r[0:1, 17 * RW : 33 * RW], s_r[126:127, 16 * RW : 32 * RW]
                )._wait_ge(act_sem, 33 * t).then_inc(h2_sem, 16)
                nc.sync.dma_start(
                    s_r[127:128, 16 * RW : 32 * RW], s_r[1:2, 17 * RW : 33 * RW]
                )._wait_ge(act_sem, 33 * t).then_inc(h2_sem, 16)
            # AS halos for the t=16 conv
            nc.sync.dma_start(
                as_r[0:1, RW : 33 * RW], as_r[126:127, 0 : 32 * RW]
            )._wait_ge(as_sem, 45).then_inc(ash_sem, 16)
            nc.sync.dma_start(
                as_r[127:128, 0 : 32 * RW], as_r[1:2, RW : 33 * RW]
            )._wait_ge(as_sem, 45).then_inc(ash_sem, 16)
            # AS out (halo partitions included; host ignores them)
            nc.sync.dma_start(asum_d[:], as_r)._wait_ge(ash_sem, 32).then_inc(
                out_sem, 16
            )
            # last out
            nc.sync.dma_start(last_d[:], stage)._wait_ge(act_sem, 33 * 15).then_inc(
                out_sem, 16
            )
            # mean out in 2 chunks as ACT(16) progresses
            nc.sync.dma_start(mean_d[:, 0:4096], mst[:, 0:4096])._wait_ge(
                act_sem, 33 * 15 + 20
            ).then_inc(out_sem, 16)
            nc.sync.dma_start(
                mean_d[:, 4096 : NB * 256], mst[:, 4096 : NB * 256]
            )._wait_ge(act_sem, 33 * 16).then_inc(out_sem, 16)
            nc.sync.wait_ge(in_sem, 16)
            nc.sync.wait_ge(h0_sem, 32 * (T - 1))
            nc.sync.wait_ge(h1_sem, 32 * (T - 1))
            nc.sync.wait_ge(h2_sem, 32 * (T - 1))
            nc.sync.wait_ge(ash_sem, 32)
            nc.sync.wait_ge(out_sem, 64)

        @block.tensor
        def _(tensor):
            for t in range(1, T + 2):  # t=16 is the gsum pass
                gsum = t == T + 1
                rhs_s = as_r if gsum else s_r
                idw = id15 if gsum else ident
                for k in range(9):
                    b0, b1 = brange(k)
                    # data-ready gating
                    if t == 1:
                        if k == 0:
                            nc.tensor.wait_ge(in_sem, 16)
                    elif t <= T:
                        if k == 0:
                            nc.tensor.wait_ge(h0_sem, 32 * (t - 1))
                        elif k == 1:
                            nc.tensor.wait_ge(h1_sem, 32 * (t - 1))
                        elif k == 4:
                            nc.tensor.wait_ge(h2_sem, 32 * (t - 1))
                    elif k == 0:
                        nc.tensor.wait_ge(ash_sem, 32)
                    # psum slot-free gating
                    if t == 1:
                        if 4 <= k <= 7:
                            nc.tensor.wait_ge(act_sem, 4 * k - 12)
                        elif k == 8:
                            nc.tensor.wait_ge(act_sem, 24)
                    else:
                        if k == 0:
                            nc.tensor.wait_ge(act_sem, 33 * (t - 2) + 20)
                        elif k == 1:
                            nc.tensor.wait_ge(act_sem, 33 * (t - 1))
                        elif k == 2:
                            nc.tensor.wait_ge(act_sem, 33 * (t - 2) + 28)
                        elif k == 3:
                            nc.tensor.wait_ge(act_sem, 33 * (t - 2) + 32)
                        elif k <= 7:
                            nc.tensor.wait_ge(act_sem, 33 * (t - 1) + 4 * (k - 4) + 4)
                        else:
                            nc.tensor.wait_ge(act_sem, 33 * (t - 1) + 24)
                    for b in range(b0, b1):
                        slot = 4 if b == 32 else b % 16
                        out = ps[:, slot * 256 : slot * 256 + 256]
                        if k in ID_BATCH:
                            nc.tensor.matmul(
                                out,
                                idw,
                                dt_r[:, b * RW + 1 : b * RW + 257],
                                start=True,
                                stop=False,
                            )
                        for j in range(3):
                            ins = nc.tensor.matmul(
                                out,
                                wv[:, (b * 3 + j) * 128 : (b * 3 + j + 1) * 128],
                                rhs_s[:, b * RW + j : b * RW + j + 256],
                                start=(k not in ID_BATCH and j == 0),
                                stop=(j == 2),
                            )
                        ins.then_inc(pe_sem, 1)

        @block.scalar
        def _(scalar):
            for t in range(1, T + 2):
                for j in range(9):
                    b0, b1 = brange(j)
                    sl = 4 if b0 == 32 else b0 % 16
                    nc.scalar.wait_ge(pe_sem, 33 * (t - 1) + (b1 if b1 < 33 else 33))
                    if 1 <= j <= 4:
                        nc.scalar.wait_ge(d_sem, 4 * (t - 1) + j)
                    if t <= T:
                        if j == 0:
                            nc.scalar.wait_ge(as_sem, 3 * (t - 1) + 1)
                        elif j == 4:
                            nc.scalar.wait_ge(as_sem, 3 * (t - 1) + 2)
                        elif j == 6:
                            nc.scalar.wait_ge(as_sem, 3 * (t - 1) + 3)
                    if t < T:
                        ins = nc.scalar.activation(
                            s_3d[:, b0:b1, 1:257],
                            ps_3d[:, sl : sl + (b1 - b0), :],
                            Act.Square,
                        )
                    elif t == T:  # last-1/2 -> f32 stage
                        ins = nc.scalar.activation(
                            st_3d[:, b0:b1, :],
                            ps_3d[:, sl : sl + (b1 - b0), :],
                            Act.Identity,
                        )
                    else:  # t=16: mean-1/2 = psum/15 -> dead s region (f16)
                        ins = nc.scalar.activation(
                            mst_3d[:, b0:b1, :],
                            ps_3d[:, sl : sl + (b1 - b0), :],
                            Act.Identity,
                            scale=float(1.0 / 15.0),
                        )
                    ins.then_inc(act_sem, b1 - b0)

        @block.vector
        def _(vector):
            # per step: AS chunks c0 (regions 0..15), c1a (16..23), c1b
            # (24..32) accumulating s_{t-1}, interleaved with Dt psum-adds
            # for blocks 4..19.  Step 16 runs only the Dt adds (scale 15).
            for t in range(1, T + 2):
                if t <= T:
                    prog = [
                        ("as", (0, 16, 1)),
                        ("dt", 0),
                        ("dt", 1),
                        ("as", (16, 24, 2)),
                        ("dt", 2),
                        ("dt", 3),
                        ("as", (24, 33, 3)),
                    ]
                else:
                    prog = [("dt", 0), ("dt", 1), ("dt", 2), ("dt", 3)]
                for kind, arg in prog:
                    if kind == "as":
                        r0, r1, ci = arg
                        if t == 1:
                            if ci == 1:
                                nc.vector.wait_ge(in_sem, 16)
                            nc.vector.tensor_scalar(
                                out=as_3d[:, r0:r1, :],
                                in0=s_3d[:, r0:r1, :],
                                scalar1=1.0,
                                scalar2=None,
                                op0=Alu.mult,
                            ).then_inc(as_sem, 1)
                        else:
                            if ci == 1:
                                nc.vector.wait_ge(act_sem, 33 * (t - 2) + 16)
                                nc.vector.wait_ge(h0_sem, 32 * (t - 1))
                                nc.vector.wait_ge(h1_sem, 32 * (t - 1))
                            elif ci == 2:
                                nc.vector.wait_ge(act_sem, 33 * (t - 2) + 24)
                                nc.vector.wait_ge(h2_sem, 32 * (t - 1))
                            else:
                                nc.vector.wait_ge(act_sem, 33 * (t - 1))
                            nc.vector.tensor_tensor(
                                out=as_3d[:, r0:r1, :],
                                in0=as_3d[:, r0:r1, :],
                                in1=s_3d[:, r0:r1, :],
                                op=Alu.add,
                            ).then_inc(as_sem, 1)
                    else:
                        b0, b1, sb = D_BATCH[arg]
                        nc.vector.wait_ge(pe_sem, 33 * (t - 1) + b1)
                        psl = ps[:, sb * 256 : (sb + (b1 - b0)) * 256]
                        nc.vector.scalar_tensor_tensor(
                            out=psl,
                            in0=dt_3d[:, b0:b1, 1:257],
                            scalar=15.0 if t == T + 1 else 1.0,
                            in1=psl,
                            op0=Alu.mult,
                            op1=Alu.add,
                        ).then_inc(d_sem, 1)

    return nc


def _get_nc():
    if "nc" not in _CACHE:
        _CACHE["nc"] = _build_program()
    return _CACHE["nc"]


def _stream_maps():
    if "maps" in _CACHE:
        return _CACHE["maps"]
    b_idx = np.arange(NB)
    p_idx = np.arange(128)
    pos = 126 * b_idx[None, :] + p_idx[:, None]  # [128, NB]
    real = (pos % RPI != 0) & (pos < NS - 1)
    img = np.where(real, pos // RPI, 0)
    row = np.where(real, pos % RPI - 1, 0)
    # inverse: (img, row) -> owning (partition, block)
    pos2 = RPI * np.arange(NIMG)[:, None] + 1 + np.arange(H)[None, :]
    ub = (pos2 - 1) // OWN
    up = pos2 - OWN * ub
    _CACHE["maps"] = (real, img, row, ub, up)
    return _CACHE["maps"]


def _conv_inbounds_sum(k):
    c0 = np.zeros((H, W), dtype=np.float64)
    pad = np.pad(np.ones((H, W), dtype=np.float64), 1)
    for dy in range(3):
        for dx in range(3):
            c0 += k[dy, dx] * pad[dy : dy + H, dx : dx + W]
    return c0


def _build_weights(k):
    """[128, NB*3*128] f16: banded negated lhsT, per-output-row channel."""
    real, img, row, _, _ = _stream_maps()
    w_eff = (np.float32(0.255 * R_PARAM) * k).astype(np.float32)  # [C,3,3]
    w_eff[:, 1, 1] += np.float32(0.595 * R_PARAM)
    ch = img % C
    wp = np.zeros((128, NB, 3, 128), dtype=np.float32)
    m = np.arange(128)
    for dy in (-1, 0, 1):
        mv = m[(m + dy >= 0) & (m + dy <= 127)]
        kv = mv + dy
        for b in range(NB):
            rm = real[mv, b]
            wp[kv[rm], b, :, mv[rm]] = -w_eff[ch[mv[rm], b], dy + 1, :]
    return wp.reshape(128, NB * 3 * 128).astype(np.float16)


def _pack_fields(field):
    """[NIMG, H, W] -> [128, NB*RW] stream-block layout (pads/Z zero), f16."""
    real, img, row, _, _ = _stream_maps()
    out = np.zeros((128, NB, RW), dtype=np.float16)
    out[:, :, 1:257] = np.where(
        real[:, :, None], field.reshape(NIMG, H, W)[img, row], np.float32(0)
    ).astype(np.float16)
    return out.reshape(128, NB * RW)


def _gather(arr, stride, off):
    """[128, NB*stride] device layout -> [NIMG, H, W] host layout."""
    _, _, _, ub, up = _stream_maps()
    return arr[up[:, :, None], ub[:, :, None] * stride + off + np.arange(W)]


def kernel(drive, K_local, trace=False):
    from concourse.bass_utils import run_bass_kernel_spmd

    drive = np.asarray(drive, dtype=np.float32)
    K_local = np.asarray(K_local, dtype=np.float32)
    k = K_local[:, 0]  # [C,3,3]

    nc = _get_nc()

    w_pack = _build_weights(k)
    ident = np.eye(128, dtype=np.float16)
    id15 = np.float16(15.0) * np.eye(128, dtype=np.float16)

    c0 = np.stack([_conv_inbounds_sum(k[c].astype(np.float64)) for c in range(C)])
    A = np.float32(R_PARAM / 4)
    d_const = (np.float32(0.595) * A + np.float32(0.255) * A * c0[None]).astype(
        np.float32
    )

    in_maps = []
    for cid in range(N_CORES):
        dcore = drive[BL * cid : BL * (cid + 1)]
        Dt = (d_const + np.float32(0.15) * dcore - np.float32(0.5)).astype(np.float32)
        s0 = np.square(dcore - np.float32(0.5), dtype=np.float32)
        inp = np.empty((128, IN_LEN), dtype=np.float16)
        inp[:, W_OFF:ID_OFF] = w_pack
        inp[:, ID_OFF:ID15_OFF] = ident
        inp[:, ID15_OFF:DT_OFF] = id15
        inp[:, DT_OFF:S_OFF] = _pack_fields(Dt.reshape(NIMG, H, W))
        inp[:, S_OFF:IN_LEN] = _pack_fields(s0.reshape(NIMG, H, W))
        in_maps.append({"inp": inp})

    r = run_bass_kernel_spmd(nc, in_maps, list(range(N_CORES)), trace=trace)
    if trace and r.exec_time_ns is not None:
        print(f"HW exec time: {r.exec_time_ns} ns")
        _CACHE["exec_time_ns"] = r.exec_time_ns
        _CACHE["profile"] = r
    res = r.results

    last = np.empty((B, C, H, W), dtype=np.float32)
    mean = np.empty((B, C, H, W), dtype=np.float32)
    var = np.empty((B, C, H, W), dtype=np.float32)
    inv15 = np.float32(1.0 / 15.0)
    for cid in range(N_CORES):
        sl = slice(BL * cid, BL * (cid + 1))
        dcore = drive[sl]
        lc = _gather(res[cid]["last_ps"], 256, 0).reshape(BL, C, H, W) + np.float32(0.5)
        mc = _gather(res[cid]["mean_ps"].astype(np.float32), 256, 0).reshape(
            BL, C, H, W
        ) + np.float32(0.5)
        ac = _gather(res[cid]["asum"].astype(np.float32), RW, 1).reshape(BL, C, H, W)
        s0 = np.square(dcore - np.float32(0.5), dtype=np.float32)
        s15 = np.square(lc - np.float32(0.5), dtype=np.float32)
        ssum15 = ac - s0 + s15
        last[sl] = lc
        mean[sl] = mc
        var[sl] = ssum15 * inv15 - np.square(mc - np.float32(0.5), dtype=np.float32)

    delta = last - drive
    return (last, mean, var, delta, delta.copy())


# revision 22
# speedup vs baseline: 5.6911x; 1.0586x over previous
"""CML2DWithStats Trainium2 kernel — PE-array (matmul) stencil, f16 operands.

15-step coupled map lattice on [16,8,256,256] f32:
    m = R*g*(1-g);  g' = clamp(0.595*m + 0.255*conv3x3(m) + 0.15*drive)
returning (last, mean, var, delta, delta).  Clamp never binds.

Reformulation (s := (g-1/2)^2, a := R/4):
    g' - 1/2 = Dt - L(s),  Dt = 0.595*a + 0.255*a*C0 + 0.15*drive - 1/2
    L = 3x3 stencil, weights W = R*(0.255*K + 0.595*center);  s' = (g'-1/2)^2
Stats are affine in accumulated s:  sum_t g_t = 15*Dt - L(AS) + 15/2 with
AS = sum_{t=0..14} s_t;  ssum = AS - s_0 + s_15.  Host finalizes mean/var.

Device mapping (per core; pure data parallel over 8 cores, 2 samples each):
the 16 images' rows form one zero-separated stream of 4113 rows packed into
NB=33 overlapping blocks of 128 rows (126 owned + 1 halo row each side).
Rows live on PARTITIONS, columns on the free axis ([128, 258] f16 regions,
zero pad cols).  Per block per step the stencil is 3 PE matmuls (one per
column shift dx) with banded 128x128 f16 lhsT (row taps, negated,
per-channel), accumulating into a PSUM slot; f16 keeps matmuls at 1 PE
cycle/row and its 10-bit mantissa leaves ~1e-3 final error (gate is 2e-2).
Dt enters via an identity matmul on PE for blocks 0..3 and 20..32, and via
DVE scalar_tensor_tensor psum-adds for blocks 4..19 (load balance: PE would
otherwise bottleneck).  ACT squares PSUM -> s (f16), DVE accumulates AS
(f16, 2x mode), 6 tiny SBUF-SBUF DMAs per step refresh halo partition rows
at three granularities so the next step's PE never stalls.  Step 15: ACT
evacuates psum = last-1/2 to an f32 stage; step 16 computes 15*Dt - L(AS)
on PE (id15 / scalar=15) and ACT evacuates psum/15 into the dead s region.
Host gathers last/mean/AS and finishes mean/var/delta.

Raw Bass; hand-counted semaphore targets; DMA ordering uses one inline
wait per DMA instruction (async transfers ignore seq-level waits) and
role-separated semaphores so thresholds are unambiguous under out-of-order
DMA completion.
"""

import sys

sys.path.insert(0, "/opt/trn_rl_repo")

import numpy as np

R_PARAM = 3.9
STEPS = 15

B, C, H, W = 16, 8, 256, 256
N_CORES = 8
BL = B // N_CORES  # samples per core
NIMG = BL * C  # 16 images per core

NB = 33  # row blocks
OWN = 126  # owned rows per block
RPI = H + 1  # stream rows per image (1 leading Z + 256)
NS = NIMG * RPI + 1  # 4113 stream rows (incl trailing Z)
RW = 258  # region width (256 cols + 2 zero pad)

ID_OFF = 0
ID15_OFF = 128
W_OFF = 256
W_LEN = NB * 3 * 128  # 12672
DT_OFF = W_OFF + W_LEN  # 12928
S_OFF = DT_OFF + NB * RW  # 21442
IN_LEN = S_OFF + NB * RW  # 29956 f16 elements DMAed in
AS_OFF = IN_LEN
BASE_F = AS_OFF + NB * RW  # 38470 f16 per partition
STAGE_F = NB * 256  # 8448 f32 per partition (last)

# identity-matmul blocks on PE; Dt for the rest is added by DVE
ID_BATCH = {0, 5, 6, 7, 8}  # blocks 0..3 and 20..32
# DVE Dt batches: (block range, psum slot base)
D_BATCH = [(4, 8, 4), (8, 12, 8), (12, 16, 12), (16, 20, 0)]

_CACHE = {}


def _build_program():
    import concourse.bass as bass
    import concourse.mybir as mybir

    f32 = mybir.dt.float32
    f16 = mybir.dt.float16
    Alu = mybir.AluOpType
    Act = mybir.ActivationFunctionType

    nc = bass.Bass()

    inp_d = nc.dram_tensor("inp", [128, IN_LEN], f16, kind="ExternalInput")
    last_d = nc.dram_tensor("last_ps", [128, NB * 256], f32, kind="ExternalOutput")
    mean_d = nc.dram_tensor("mean_ps", [128, NB * 256], f16, kind="ExternalOutput")
    asum_d = nc.dram_tensor("asum", [128, NB * RW], f16, kind="ExternalOutput")

    base_t = nc.alloc_sbuf_tensor("base", [128, BASE_F], f16)
    stage_t = nc.alloc_sbuf_tensor("stage", [128, STAGE_F], f32)
    psum_t = nc.alloc_psum_tensor("ps", [128, 4096], f32)

    base = base_t.ap()
    ps = psum_t.ap()

    wv = base[:, W_OFF : W_OFF + W_LEN]
    ident = base[:, ID_OFF : ID_OFF + 128]
    id15 = base[:, ID15_OFF : ID15_OFF + 128]
    dt_r = base[:, DT_OFF : DT_OFF + NB * RW]
    s_r = base[:, S_OFF : S_OFF + NB * RW]
    as_r = base[:, AS_OFF : AS_OFF + NB * RW]
    stage = stage_t.ap()
    mst = s_r[:, 0 : NB * 256]

    dt_3d = dt_r.rearrange("p (b x) -> p b x", x=RW)
    s_3d = s_r.rearrange("p (b x) -> p b x", x=RW)
    as_3d = as_r.rearrange("p (b x) -> p b x", x=RW)
    st_3d = stage.rearrange("p (b x) -> p b x", x=256)
    mst_3d = mst.rearrange("p (b x) -> p b x", x=256)
    ps_3d = ps.rearrange("p (s x) -> p s x", x=256)

    def brange(j):
        return (4 * j, min(4 * j + 4, NB))

    T = STEPS

    with (
        nc.semaphore() as in_sem,
        nc.semaphore() as g1_sem,
        nc.semaphore() as g2_sem,
        nc.semaphore() as h0_sem,
        nc.semaphore() as h1_sem,
        nc.semaphore() as h2_sem,
        nc.semaphore() as ash_sem,
        nc.semaphore() as out_sem,
        nc.semaphore() as pe_sem,
        nc.semaphore() as act_sem,
        nc.semaphore() as as_sem,
        nc.semaphore() as d_sem,
        nc.Block() as block,
    ):

        @block.sync
        def _(sync):
            # input in 3 block-groups (x3 spans each) so PE can start early
            for (b0, b1), sem in zip(
                ((0, 12), (12, 24), (24, 33)), (in_sem, g1_sem, g2_sem)
            ):
                w0 = 0 if b0 == 0 else W_OFF + b0 * 384
                nc.sync.dma_start(
                    base[:, w0 : W_OFF + b1 * 384], inp_d[:, w0 : W_OFF + b1 * 384]
                ).then_inc(sem, 16)
                nc.sync.dma_start(
                    base[:, DT_OFF + b0 * RW : DT_OFF + b1 * RW],
                    inp_d[:, DT_OFF + b0 * RW : DT_OFF + b1 * RW],
                ).then_inc(sem, 16)
                nc.sync.dma_start(
                    base[:, S_OFF + b0 * RW : S_OFF + b1 * RW],
                    inp_d[:, S_OFF + b0 * RW : S_OFF + b1 * RW],
                ).then_inc(sem, 16)
            for t in range(1, T):  # halos completing s_t, t = 1..14
                # pair 0: dst regions 0..4 (after ACT(t) blocks 0..7)
                nc.sync.dma_start(
                    s_r[0:1, RW : 5 * RW], s_r[126:127, 0 : 4 * RW]
                )._wait_ge(act_sem, 33 * (t - 1) + 8).then_inc(h0_sem, 16)
                nc.sync.dma_start(
                    s_r[127:128, 0 : 4 * RW], s_r[1:2, RW : 5 * RW]
                )._wait_ge(act_sem, 33 * (t - 1) + 8).then_inc(h0_sem, 16)
                # pair 1: dst regions 4..16 (after ACT(t) blocks 0..19)
                nc.sync.dma_start(
                    s_r[0:1, 5 * RW : 17 * RW], s_r[126:127, 4 * RW : 16 * RW]
                )._wait_ge(act_sem, 33 * (t - 1) + 20).then_inc(h1_sem, 16)
                nc.sync.dma_start(
                    s_r[127:128, 4 * RW : 16 * RW], s_r[1:2, 5 * RW : 17 * RW]
                )._wait_ge(act_sem, 33 * (t - 1) + 20).then_inc(h1_sem, 16)
                # pair 2: dst regions 16..32 (after all ACT(t))
                nc.sync.dma_start(
                    s_r[0:1, 17 * RW : 33 * RW], s_r[126:127, 16 * RW : 32 * RW]
                )._wait_ge(act_sem, 33 * t).then_inc(h2_sem, 16)
                nc.sync.dma_start(
                    s_r[127:128, 16 * RW : 32 * RW], s_r[1:2, 17 * RW : 33 * RW]
                )._wait_ge(act_sem, 33 * t).then_inc(h2_sem, 16)
            # AS halos for the t=16 conv
            nc.sync.dma_start(
                as_r[0:1, RW : 33 * RW], as_r[126:127, 0 : 32 * RW]
            )._wait_ge(as_sem, 45).then_inc(ash_sem, 16)
            nc.sync.dma_start(
                as_r[127:128, 0 : 32 * RW], as_r[1:2, RW : 33 * RW]
            )._wait_ge(as_sem, 45).then_inc(ash_sem, 16)
            # AS out (halo partitions included; host ignores them)
            nc.sync.dma_start(asum_d[:], as_r)._wait_ge(ash_sem, 32).then_inc(
                out_sem, 16
            )
            # last out
            nc.sync.dma_start(last_d[:], stage)._wait_ge(act_sem, 33 * 15).then_inc(
                out_sem, 16
            )
            # mean out in 2 chunks as ACT(16) progresses
            nc.sync.dma_start(mean_d[:, 0:4096], mst[:, 0:4096])._wait_ge(
                act_sem, 33 * 15 + 20
            ).then_inc(out_sem, 16)
            nc.sync.dma_start(
                mean_d[:, 4096 : NB * 256], mst[:, 4096 : NB * 256]
            )._wait_ge(act_sem, 33 * 16).then_inc(out_sem, 16)
            nc.sync.wait_ge(in_sem, 48)
            nc.sync.wait_ge(g1_sem, 48)
            nc.sync.wait_ge(g2_sem, 48)
            nc.sync.wait_ge(h0_sem, 32 * (T - 1))
            nc.sync.wait_ge(h1_sem, 32 * (T - 1))
            nc.sync.wait_ge(h2_sem, 32 * (T - 1))
            nc.sync.wait_ge(ash_sem, 32)
            nc.sync.wait_ge(out_sem, 64)

        @block.tensor
        def _(tensor):
            for t in range(1, T + 2):  # t=16 is the gsum pass
                gsum = t == T + 1
                rhs_s = as_r if gsum else s_r
                idw = id15 if gsum else ident
                for k in range(9):
                    b0, b1 = brange(k)
                    # data-ready gating
                    if t == 1:
                        if k == 0:
                            nc.tensor.wait_ge(in_sem, 48)
                        elif k == 3:
                            nc.tensor.wait_ge(g1_sem, 48)
                        elif k == 6:
                            nc.tensor.wait_ge(g2_sem, 48)
                    elif t <= T:
                        if k == 0:
                            nc.tensor.wait_ge(h0_sem, 32 * (t - 1))
                        elif k == 1:
                            nc.tensor.wait_ge(h1_sem, 32 * (t - 1))
                        elif k == 4:
                            nc.tensor.wait_ge(h2_sem, 32 * (t - 1))
                    elif k == 0:
                        nc.tensor.wait_ge(ash_sem, 32)
                    # psum slot-free gating
                    if t == 1:
                        if 4 <= k <= 7:
                            nc.tensor.wait_ge(act_sem, 4 * k - 12)
                        elif k == 8:
                            nc.tensor.wait_ge(act_sem, 24)
                    else:
                        if k == 0:
                            nc.tensor.wait_ge(act_sem, 33 * (t - 2) + 20)
                        elif k == 1:
                            nc.tensor.wait_ge(act_sem, 33 * (t - 1))
                        elif k == 2:
                            nc.tensor.wait_ge(act_sem, 33 * (t - 2) + 28)
                        elif k == 3:
                            nc.tensor.wait_ge(act_sem, 33 * (t - 2) + 32)
                        elif k <= 7:
                            nc.tensor.wait_ge(act_sem, 33 * (t - 1) + 4 * (k - 4) + 4)
                        else:
                            nc.tensor.wait_ge(act_sem, 33 * (t - 1) + 24)
                    for b in range(b0, b1):
                        slot = 4 if b == 32 else b % 16
                        out = ps[:, slot * 256 : slot * 256 + 256]
                        if k in ID_BATCH:
                            nc.tensor.matmul(
                                out,
                                idw,
                                dt_r[:, b * RW + 1 : b * RW + 257],
                                start=True,
                                stop=False,
                            )
                        for j in range(3):
                            ins = nc.tensor.matmul(
                                out,
                                wv[:, (b * 3 + j) * 128 : (b * 3 + j + 1) * 128],
                                rhs_s[:, b * RW + j : b * RW + j + 256],
                                start=(k not in ID_BATCH and j == 0),
                                stop=(j == 2),
                            )
                        ins.then_inc(pe_sem, 1)

        @block.scalar
        def _(scalar):
            for t in range(1, T + 2):
                for j in range(9):
                    b0, b1 = brange(j)
                    sl = 4 if b0 == 32 else b0 % 16
                    nc.scalar.wait_ge(pe_sem, 33 * (t - 1) + (b1 if b1 < 33 else 33))
                    if 1 <= j <= 4:
                        nc.scalar.wait_ge(d_sem, 4 * (t - 1) + j)
                    if t == 1:
                        if j == 0:
                            nc.scalar.wait_ge(as_sem, 1)
                        elif j == 3:
                            nc.scalar.wait_ge(as_sem, 2)
                        elif j == 6:
                            nc.scalar.wait_ge(as_sem, 3)
                    elif t <= T:
                        if j == 0:
                            nc.scalar.wait_ge(as_sem, 3 * (t - 1) + 1)
                        elif j == 4:
                            nc.scalar.wait_ge(as_sem, 3 * (t - 1) + 2)
                        elif j == 6:
                            nc.scalar.wait_ge(as_sem, 3 * (t - 1) + 3)
                    if t < T:
                        ins = nc.scalar.activation(
                            s_3d[:, b0:b1, 1:257],
                            ps_3d[:, sl : sl + (b1 - b0), :],
                            Act.Square,
                        )
                    elif t == T:  # last-1/2 -> f32 stage
                        ins = nc.scalar.activation(
                            st_3d[:, b0:b1, :],
                            ps_3d[:, sl : sl + (b1 - b0), :],
                            Act.Identity,
                        )
                    else:  # t=16: mean-1/2 = psum/15 -> dead s region (f16)
                        ins = nc.scalar.activation(
                            mst_3d[:, b0:b1, :],
                            ps_3d[:, sl : sl + (b1 - b0), :],
                            Act.Identity,
                            scale=float(1.0 / 15.0),
                        )
                    ins.then_inc(act_sem, b1 - b0)

        @block.vector
        def _(vector):
            # per step: AS chunks c0 (regions 0..15), c1a (16..23), c1b
            # (24..32) accumulating s_{t-1}, interleaved with Dt psum-adds
            # for blocks 4..19.  Step 16 runs only the Dt adds (scale 15).
            for t in range(1, T + 2):
                if t == 1:
                    prog = [
                        ("as", (0, 12, 1)),
                        ("dt", 0),
                        ("dt", 1),
                        ("as", (12, 24, 2)),
                        ("dt", 2),
                        ("dt", 3),
                        ("as", (24, 33, 3)),
                    ]
                elif t <= T:
                    prog = [
                        ("as", (0, 16, 1)),
                        ("dt", 0),
                        ("dt", 1),
                        ("as", (16, 24, 2)),
                        ("dt", 2),
                        ("dt", 3),
                        ("as", (24, 33, 3)),
                    ]
                else:
                    prog = [("dt", 0), ("dt", 1), ("dt", 2), ("dt", 3)]
                for kind, arg in prog:
                    if kind == "as":
                        r0, r1, ci = arg
                        if t == 1:
                            if ci == 1:
                                nc.vector.wait_ge(in_sem, 48)
                            elif ci == 2:
                                nc.vector.wait_ge(g1_sem, 48)
                            else:
                                nc.vector.wait_ge(g2_sem, 48)
                            nc.vector.tensor_scalar(
                                out=as_3d[:, r0:r1, :],
                                in0=s_3d[:, r0:r1, :],
                                scalar1=1.0,
                                scalar2=None,
                                op0=Alu.mult,
                            ).then_inc(as_sem, 1)
                        else:
                            if ci == 1:
                                nc.vector.wait_ge(act_sem, 33 * (t - 2) + 16)
                                nc.vector.wait_ge(h0_sem, 32 * (t - 1))
                                nc.vector.wait_ge(h1_sem, 32 * (t - 1))
                            elif ci == 2:
                                nc.vector.wait_ge(act_sem, 33 * (t - 2) + 24)
                                nc.vector.wait_ge(h2_sem, 32 * (t - 1))
                            else:
                                nc.vector.wait_ge(act_sem, 33 * (t - 1))
                            nc.vector.tensor_tensor(
                                out=as_3d[:, r0:r1, :],
                                in0=as_3d[:, r0:r1, :],
                                in1=s_3d[:, r0:r1, :],
                                op=Alu.add,
                            ).then_inc(as_sem, 1)
                    else:
                        b0, b1, sb = D_BATCH[arg]
                        nc.vector.wait_ge(pe_sem, 33 * (t - 1) + b1)
                        psl = ps[:, sb * 256 : (sb + (b1 - b0)) * 256]
                        nc.vector.scalar_tensor_tensor(
                            out=psl,
                            in0=dt_3d[:, b0:b1, 1:257],
                            scalar=15.0 if t == T + 1 else 1.0,
                            in1=psl,
                            op0=Alu.mult,
                            op1=Alu.add,
                        ).then_inc(d_sem, 1)

    return nc


def _get_nc():
    if "nc" not in _CACHE:
        _CACHE["nc"] = _build_program()
    return _CACHE["nc"]


def _stream_maps():
    if "maps" in _CACHE:
        return _CACHE["maps"]
    b_idx = np.arange(NB)
    p_idx = np.arange(128)
    pos = 126 * b_idx[None, :] + p_idx[:, None]  # [128, NB]
    real = (pos % RPI != 0) & (pos < NS - 1)
    img = np.where(real, pos // RPI, 0)
    row = np.where(real, pos % RPI - 1, 0)
    # inverse: (img, row) -> owning (partition, block)
    pos2 = RPI * np.arange(NIMG)[:, None] + 1 + np.arange(H)[None, :]
    ub = (pos2 - 1) // OWN
    up = pos2 - OWN * ub
    _CACHE["maps"] = (real, img, row, ub, up)
    return _CACHE["maps"]


def _conv_inbounds_sum(k):
    c0 = np.zeros((H, W), dtype=np.float64)
    pad = np.pad(np.ones((H, W), dtype=np.float64), 1)
    for dy in range(3):
        for dx in range(3):
            c0 += k[dy, dx] * pad[dy : dy + H, dx : dx + W]
    return c0


def _build_weights(k):
    """[128, NB*3*128] f16: banded negated lhsT, per-output-row channel."""
    real, img, row, _, _ = _stream_maps()
    w_eff = (np.float32(0.255 * R_PARAM) * k).astype(np.float32)  # [C,3,3]
    w_eff[:, 1, 1] += np.float32(0.595 * R_PARAM)
    ch = img % C
    wp = np.zeros((128, NB, 3, 128), dtype=np.float32)
    m = np.arange(128)
    for dy in (-1, 0, 1):
        mv = m[(m + dy >= 0) & (m + dy <= 127)]
        kv = mv + dy
        for b in range(NB):
            rm = real[mv, b]
            wp[kv[rm], b, :, mv[rm]] = -w_eff[ch[mv[rm], b], dy + 1, :]
    return wp.reshape(128, NB * 3 * 128).astype(np.float16)


def _pack_fields(field):
    """[NIMG, H, W] -> [128, NB*RW] stream-block layout (pads/Z zero), f16."""
    real, img, row, _, _ = _stream_maps()
    out = np.zeros((128, NB, RW), dtype=np.float16)
    out[:, :, 1:257] = np.where(
        real[:, :, None], field.reshape(NIMG, H, W)[img, row], np.float32(0)
    ).astype(np.float16)
    return out.reshape(128, NB * RW)


def _gather(arr, stride, off):
    """[128, NB*stride] device layout -> [NIMG, H, W] host layout."""
    _, _, _, ub, up = _stream_maps()
    return arr[up[:, :, None], ub[:, :, None] * stride + off + np.arange(W)]


def kernel(drive, K_local, trace=False):
    from concourse.bass_utils import run_bass_kernel_spmd

    drive = np.asarray(drive, dtype=np.float32)
    K_local = np.asarray(K_local, dtype=np.float32)
    k = K_local[:, 0]  # [C,3,3]

    nc = _get_nc()

    w_pack = _build_weights(k)
    ident = np.eye(128, dtype=np.float16)
    id15 = np.float16(15.0) * np.eye(128, dtype=np.float16)

    c0 = np.stack([_conv_inbounds_sum(k[c].astype(np.float64)) for c in range(C)])
    A = np.float32(R_PARAM / 4)
    d_const = (np.float32(0.595) * A + np.float32(0.255) * A * c0[None]).astype(
        np.float32
    )

    in_maps = []
    for cid in range(N_CORES):
        dcore = drive[BL * cid : BL * (cid + 1)]
        Dt = (d_const + np.float32(0.15) * dcore - np.float32(0.5)).astype(np.float32)
        s0 = np.square(dcore - np.float32(0.5), dtype=np.float32)
        inp = np.empty((128, IN_LEN), dtype=np.float16)
        inp[:, ID_OFF:ID15_OFF] = ident
        inp[:, ID15_OFF:W_OFF] = id15
        inp[:, W_OFF:DT_OFF] = w_pack
        inp[:, DT_OFF:S_OFF] = _pack_fields(Dt.reshape(NIMG, H, W))
        inp[:, S_OFF:IN_LEN] = _pack_fields(s0.reshape(NIMG, H, W))
        in_maps.append({"inp": inp})

    r = run_bass_kernel_spmd(nc, in_maps, list(range(N_CORES)), trace=trace)
    if trace and r.exec_time_ns is not None:
        print(f"HW exec time: {r.exec_time_ns} ns")
        _CACHE["exec_time_ns"] = r.exec_time_ns
        _CACHE["profile"] = r
    res = r.results

    last = np.empty((B, C, H, W), dtype=np.float32)
    mean = np.empty((B, C, H, W), dtype=np.float32)
    var = np.empty((B, C, H, W), dtype=np.float32)
    inv15 = np.float32(1.0 / 15.0)
    for cid in range(N_CORES):
        sl = slice(BL * cid, BL * (cid + 1))
        dcore = drive[sl]
        lc = _gather(res[cid]["last_ps"], 256, 0).reshape(BL, C, H, W) + np.float32(0.5)
        mc = _gather(res[cid]["mean_ps"].astype(np.float32), 256, 0).reshape(
            BL, C, H, W
        ) + np.float32(0.5)
        ac = _gather(res[cid]["asum"].astype(np.float32), RW, 1).reshape(BL, C, H, W)
        s0 = np.square(dcore - np.float32(0.5), dtype=np.float32)
        s15 = np.square(lc - np.float32(0.5), dtype=np.float32)
        ssum15 = ac - s0 + s15
        last[sl] = lc
        mean[sl] = mc
        var[sl] = ssum15 * inv15 - np.square(mc - np.float32(0.5), dtype=np.float32)

    delta = last - drive
    return (last, mean, var, delta, delta.copy())
